# revision 16
# baseline (speedup 1.0000x reference)
"""Trainium2 Bass kernel for nn_ChannelAttention (B=4, C=256, nh=8, N=24^3).

Sharding: 8 cores = 4 batches x 2 token-halves. Each core computes ALL 256
output channels for its 6912 tokens (no collectives; identical program).

Key identity: the d x d channel-attention logits only need the C x C token
Gram of x:  H = Wk^T (x x^T) Wq,  ||q_d||^2 = diag(Wq^T Gx Wq),
||k_e||^2 = diag(Wk^T Gx Wk).  So phase 1 streams x once in fp8 (tokens on
partitions, DoubleRow K=256) accumulating Gx in PSUM, and the whole
q/k-projection + Gram of the baseline collapses into tiny [256,256] matmuls.
The softmax'd attention A (block-diag, 8 heads x 32) and the 1/Z row scale
are then folded into the v-weights:  Weff = Wv A_scaled^T, so phase 2 is a
single streamed projection out = Weff^T x from a bf16 channels-major shard.

Per-core DMA: x8 tok-major full-N (3.54MB) + xbf ch-major half-N (3.54MB)
+ out bf16 (3.54MB) = 10.6MB vs 14.2MB baseline; PE ~49k cycles.

Gx must cover all N tokens (cosines need the full reduction), hence the
full-N fp8 upload; everything else is sharded by token-half.
"""

import os

import numpy as np
import ml_dtypes

BF16 = ml_dtypes.bfloat16
FP8 = ml_dtypes.float8_e4m3
P = 128
C = 256
NH = 8
N = 24 * 24 * 24  # 13824
NHALF = N // 2  # 6912
B = 4
NCORES = 8
EPS = 1e-12
NPAIRS = N // 256  # 54 fp8 DoubleRow token-pairs for Gx
CHUNK2 = 512
# phase-2 chunks over the 6912-token shard
P2CHUNKS = [CHUNK2] * 13 + [256]
# x8 slabs (token units, multiples of 256); first small so Gx starts early
X8SLABS = [512] + [1664] * 8
XBFSLABS = [1152] * 6

_PROGRAM_CACHE = {}
LAST_RESULTS = None  # test harness reads exec_time_ns from here


def _build_program():
    import concourse.mybir as mybir
    from concourse import bacc

    # Bias the act-table picker: the only funcs this kernel uses are
    # {Copy, Ln, Exp}. One real set (natural_log_exp_and_others) contains all
    # three, but the greedy picker matches the first set per func, splitting
    # them across two sets (mid-kernel 1.3us loads). Strip ln/exp from every
    # other set (ids are positional, so order/length must not change) so the
    # whole kernel runs off a single preloaded set.
    _orig_tables = bacc.get_activation_tables

    def _patched_tables(arch):
        tabs = _orig_tables(arch)
        ln = mybir.ActivationFunctionType.Ln
        ex = mybir.ActivationFunctionType.Exp
        combined = {
            name for name, funcs in tabs.items() if ln in funcs and ex in funcs
        }
        if combined:
            keep = next(iter(combined))
            tabs = {
                name: (funcs if name == keep else funcs - {ln, ex})
                for name, funcs in tabs.items()
            }
        return tabs

    bacc.get_activation_tables = _patched_tables
    try:
        return _build_program_inner(
            nc_factory=lambda: bacc.Bacc("TRN2", target_bir_lowering=False)
        )
    finally:
        bacc.get_activation_tables = _orig_tables


def _build_program_inner(nc_factory):
    import concourse.mybir as mybir
    import concourse.tile as tile

    f32 = mybir.dt.float32
    bf = mybir.dt.bfloat16
    f8 = mybir.dt.float8e4
    AF = mybir.ActivationFunctionType
    DR = mybir.MatmulPerfMode.DoubleRow

    nc = nc_factory()

    # DRAM tensors.
    # x8t: fp8, tokens-on-partitions, FULL N. free index f = 256*j + cc with
    #   token t = 256*j + 128*ko + p, channel cc.
    x8t_d = nc.dram_tensor("x8t", [P, 2, N], f8, kind="ExternalInput")
    # xbf: bf16, channels-on-partitions, my half. [p, ch, n] = x[128*ch+p, n]
    xbf_d = nc.dram_tensor("xbf", [P, 2, NHALF], bf, kind="ExternalInput")
    # wq/wk: [p, h2, d] = W[128*h2+p, d]; wvt: [p, eh, c] = Wv[c, 128*eh+p]
    wq_d = nc.dram_tensor("wq", [P, 2, C], bf, kind="ExternalInput")
    wk_d = nc.dram_tensor("wk", [P, 2, C], bf, kind="ExternalInput")
    wvt_d = nc.dram_tensor("wvt", [P, 2, C], bf, kind="ExternalInput")
    # consts: ident(128) | maskA(256) | maskB(256) | tempA(1) | tempB(1)
    consts_d = nc.dram_tensor("consts", [P, 642], f32, kind="ExternalInput")
    # out: [p, dh, n] = out[128*dh+p, n], bf16 (host upcasts)
    out_d = nc.dram_tensor("out", [P, 2, NHALF], bf, kind="ExternalOutput")

    with tile.TileContext(nc) as tc:
        with tc.tile_pool(name="persist", bufs=1) as persist:
            x8t = persist.tile([P, 2, N], f8)
            xbf = persist.tile([P, 2, NHALF], bf)
            wq = persist.tile([P, 2, C], bf)
            wk = persist.tile([P, 2, C], bf)
            wvt = persist.tile([P, 2, C], bf)
            consts = persist.tile([P, 642], f32)
            onesr = persist.tile([1, P], f32)  # lhsT for K=1 row-replication
            onescl = persist.tile([P, 1], bf)  # lhsT for partition colsums
            dum0 = persist.tile([P, 1], f32)
            dum1 = persist.tile([P, 1], f32)
            # chain results consumed by phase 2
            gx_sb = persist.tile([P, 2, C], bf)
            t1_sb = persist.tile([P, 2, C], bf)
            t2_sb = persist.tile([P, 2, C], bf)
            wqt1 = persist.tile([P, 2, C], bf)
            wkt2 = persist.tile([P, 2, C], bf)
            wqs = persist.tile([P, C], bf)
            wks = persist.tile([P, C], bf)
            ems = persist.tile([P, C], bf)
            emt = persist.tile([P, 2, C], bf)  # [e%128, eh, d] masked exp
            weff_sb = persist.tile([P, 2, C], bf)  # [c%128, h, d], 1/Z folded
            s_sb = persist.tile([P, 2, C], f32)
            e_sb = persist.tile([P, 2, C], f32)
            qn2r = persist.tile([1, C], f32)
            invqr = persist.tile([1, C], f32)
            invzr = persist.tile([1, C], f32)
            lnq = persist.tile([1, C], f32)
            kn2c = persist.tile([P, 2], f32)
            lnkc = persist.tile([P, 2], f32)
            invkc = persist.tile([P, 2], f32)
            invkt = persist.tile([P, 2], f32)
            rep_q = persist.tile([P, C], f32)
            rep_z = persist.tile([P, C], f32)

            ident = consts[:, 0:P]
            maskA = consts[:, P : P + C]
            maskB = consts[:, P + C : P + 2 * C]
            tempc = consts[:, P + 2 * C : P + 2 * C + 2]

            # constants + ACT table preload ({ln, exp, copy} set) at t=0
            nc.vector.memset(onesr, 1.0)
            nc.vector.memset(onescl, 1.0)
            nc.vector.memset(dum0, 1.0)
            nc.scalar.activation(dum1, dum0, AF.Ln)
            nc.scalar.activation(dum1, dum0, AF.Exp)

            # DMA order: first Gx pair needs x8t slab 0 only.
            edges = [0]
            for s in X8SLABS:
                edges.append(edges[-1] + s)
            nc.sync.dma_start(x8t[:, :, 0 : edges[1]], x8t_d[:, :, 0 : edges[1]])
            nc.sync.dma_start(wq, wq_d[:])
            nc.sync.dma_start(wk, wk_d[:])
            nc.sync.dma_start(wvt, wvt_d[:])
            nc.sync.dma_start(consts, consts_d[:])
            for s in range(1, len(X8SLABS)):
                nc.sync.dma_start(
                    x8t[:, :, edges[s] : edges[s + 1]],
                    x8t_d[:, :, edges[s] : edges[s + 1]],
                )
            bedges = [0]
            for s in XBFSLABS:
                bedges.append(bedges[-1] + s)
            for s in range(len(XBFSLABS)):
                nc.sync.dma_start(
                    xbf[:, :, bedges[s] : bedges[s + 1]],
                    xbf_d[:, :, bedges[s] : bedges[s + 1]],
                )

            # ---- phase 1: Gx = x x^T over all N (fp8 DoubleRow) ----
            # two tiles (= two PSUM banks): concurrent accumulation groups
            # must not share a bank (the second start re-arms bank overwrite)
            with tc.tile_pool(name="gxp", bufs=1, space="PSUM") as gxp:
                gx_ps0 = gxp.tile([P, C], f32)
                gx_ps1 = gxp.tile([P, C], f32)
                gx_ps = [gx_ps0, gx_ps1]
                for j in range(NPAIRS):
                    n0 = j * 256
                    st, sp = j == 0, j == NPAIRS - 1
                    for h1 in range(2):
                        nc.tensor.matmul(
                            gx_ps[h1],
                            x8t[:, :, n0 + 128 * h1 : n0 + 128 * h1 + 128],
                            x8t[:, :, n0 : n0 + 256],
                            start=st,
                            stop=sp,
                            perf_mode=DR,
                            skip_group_check=True,
                        )
                nc.scalar.activation(gx_sb[:, 0, :], gx_ps[0], AF.Copy)
                nc.scalar.activation(gx_sb[:, 1, :], gx_ps[1], AF.Copy)

            # ---- chain: T1/T2, H, norms, softmax, Weff fold (all tiny) ----
            with tc.tile_pool(name="chp1", bufs=1, space="PSUM") as chp1:
                t1_ps = chp1.tile([P, 2, C], f32)
                t2_ps = chp1.tile([P, 2, C], f32)

                # T1 = Gx @ Wq, T2 = Gx @ Wk  (Gx symmetric)
                for h1 in range(2):
                    for h2 in range(2):
                        nc.tensor.matmul(
                            t1_ps[:, h1, :],
                            gx_sb[:, h2, 128 * h1 : 128 * h1 + 128],
                            wq[:, h2, :],
                            start=h2 == 0,
                            stop=h2 == 1,
                            skip_group_check=True,
                        )
                        nc.tensor.matmul(
                            t2_ps[:, h1, :],
                            gx_sb[:, h2, 128 * h1 : 128 * h1 + 128],
                            wk[:, h2, :],
                            start=h2 == 0,
                            stop=h2 == 1,
                            skip_group_check=True,
                        )
                nc.scalar.activation(t1_sb, t1_ps, AF.Copy)
                nc.scalar.activation(t2_sb, t2_ps, AF.Copy)

            with tc.tile_pool(name="chp2", bufs=1, space="PSUM") as chp2:
                h_ps = chp2.tile([P, 2, C], f32)
                qn2_ps = chp2.tile([1, C], f32)
                kcol_ps0 = chp2.tile([P, 1], f32)
                kcol_ps1 = chp2.tile([P, 1], f32)
                kcol_ps = [kcol_ps0, kcol_ps1]
                repq_ps = chp2.tile([P, C], f32)

                # H[e,d] = sum_c Wk[c,e] T1[c,d]
                for eh in range(2):
                    for h2 in range(2):
                        nc.tensor.matmul(
                            h_ps[:, eh, :],
                            wk[:, h2, 128 * eh : 128 * eh + 128],
                            t1_sb[:, h2, :],
                            start=h2 == 0,
                            stop=h2 == 1,
                            skip_group_check=True,
                        )
                # qn2[d] = sum_c Wq[c,d]*T1[c,d] (row); kn2[e] as columns.
                # Pre-add the two c-halves on DVE so each PSUM reduction is a
                # single-shot matmul (no overlapping accumulation groups).
                nc.vector.tensor_mul(wqt1, wq, t1_sb)
                nc.vector.tensor_mul(wkt2, wk, t2_sb)
                nc.vector.tensor_add(wqs, wqt1[:, 0, :], wqt1[:, 1, :])
                nc.vector.tensor_add(wks, wkt2[:, 0, :], wkt2[:, 1, :])
                nc.tensor.matmul(qn2_ps, onescl, wqs, start=True, stop=True)
                for eh in range(2):
                    nc.tensor.matmul(
                        kcol_ps[eh],
                        wks[:, 128 * eh : 128 * eh + 128],
                        onescl,
                        start=True,
                        stop=True,
                    )
                nc.scalar.activation(qn2r, qn2_ps, AF.Copy)

                # invq row: 1/max(sqrt(qn2),EPS) = exp(-0.5 ln(max(qn2,EPS^2)))
                nc.vector.tensor_scalar_max(qn2r, qn2r, EPS * EPS)
                nc.scalar.activation(lnq, qn2r, AF.Ln)
                nc.scalar.activation(invqr, lnq, AF.Exp, scale=-0.5)
                # replicate invq across partitions: rep_q[p, d] = invq[d]
                nc.tensor.matmul(repq_ps, onesr, invqr, start=True, stop=True)
                nc.scalar.activation(rep_q, repq_ps, AF.Copy)

                # invk as per-partition columns
                for eh in range(2):
                    nc.vector.tensor_scalar_max(
                        kn2c[:, eh : eh + 1], kcol_ps[eh], EPS * EPS
                    )
                nc.scalar.activation(lnkc, kn2c, AF.Ln)
                nc.scalar.activation(invkc, lnkc, AF.Exp, scale=-0.5)
                nc.vector.tensor_mul(invkt, invkc, tempc)

                # S = H * rep_q; E = exp(S * invk*temp); emt = E * mask
                for eh in range(2):
                    nc.vector.tensor_mul(s_sb[:, eh, :], h_ps[:, eh, :], rep_q)
                    nc.scalar.activation(
                        e_sb[:, eh, :],
                        s_sb[:, eh, :],
                        AF.Exp,
                        scale=invkt[:, eh : eh + 1],
                    )
                nc.vector.tensor_mul(emt[:, 0, :], e_sb[:, 0, :], maskA)
                nc.vector.tensor_mul(emt[:, 1, :], e_sb[:, 1, :], maskB)

            with tc.tile_pool(name="chp3", bufs=1, space="PSUM") as chp3:
                z_ps = chp3.tile([1, C], f32)
                repz_ps = chp3.tile([P, C], f32)
                weff_ps = chp3.tile([P, 2, C], f32)

                # Z[d] = sum_e emt[e,d]; rep_z[p,d] = 1/Z[d]
                nc.vector.tensor_add(ems, emt[:, 0, :], emt[:, 1, :])
                nc.tensor.matmul(z_ps, onescl, ems, start=True, stop=True)
                nc.vector.reciprocal(invzr, z_ps)
                nc.tensor.matmul(repz_ps, onesr, invzr, start=True, stop=True)
                nc.scalar.activation(rep_z, repz_ps, AF.Copy)

                # Weff[c,d] = sum_e Wv[c,e] emt[e,d], scaled by 1/Z[d]
                for h1 in range(2):
                    for eh in range(2):
                        nc.tensor.matmul(
                            weff_ps[:, h1, :],
                            wvt[:, eh, 128 * h1 : 128 * h1 + 128],
                            emt[:, eh, :],
                            start=eh == 0,
                            stop=eh == 1,
                            skip_group_check=True,
                        )
                for h1 in range(2):
                    nc.vector.tensor_mul(
                        weff_sb[:, h1, :], weff_ps[:, h1, :], rep_z
                    )

            # ---- phase 2: out = Weff^T x (bf16 stream) ----
            with (
                tc.tile_pool(name="p2s", bufs=4) as p2s,
                tc.tile_pool(name="p2p", bufs=4, space="PSUM") as p2p,
            ):
                n0 = 0
                for jj, w in enumerate(P2CHUNKS):
                    for dh in range(2):
                        o_ps = p2p.tile([P, CHUNK2], f32, tag=f"o{dh}", bufs=2)
                        for ch in range(2):
                            nc.tensor.matmul(
                                o_ps[:, 0:w],
                                weff_sb[:, ch, 128 * dh : 128 * dh + 128],
                                xbf[:, ch, n0 : n0 + w],
                                start=ch == 0,
                                stop=ch == 1,
                                skip_group_check=True,
                            )
                        o_sb = p2s.tile([P, CHUNK2], bf, tag=f"ob{dh}", bufs=2)
                        if dh == 0:
                            nc.scalar.activation(
                                o_sb[:, 0:w], o_ps[:, 0:w], AF.Copy
                            )
                        else:
                            nc.vector.tensor_copy(o_sb[:, 0:w], o_ps[:, 0:w])
                        nc.sync.dma_start(
                            out_d[:, dh, n0 : n0 + w], o_sb[:, 0:w]
                        )
                    n0 += w

    nc.compile()
    return nc


def _get_program():
    if "nc" not in _PROGRAM_CACHE:
        _PROGRAM_CACHE["nc"] = _build_program()
    return _PROGRAM_CACHE["nc"]


def kernel(x, W_qkvv, temperature):
    global LAST_RESULTS
    from concourse.bass_utils import run_bass_kernel_spmd

    x = np.asarray(x, dtype=np.float32)
    W = np.asarray(W_qkvv, dtype=np.float32)
    temp = np.asarray(temperature, dtype=np.float32).reshape(NH)

    ident = np.eye(P, dtype=np.float32)
    mask = np.kron(np.eye(NH, dtype=np.float32), np.ones((32, 32), np.float32))
    tempv = np.repeat(temp, 32)  # [256]
    consts = np.concatenate(
        [
            ident,
            mask[0:128, :],
            mask[128:256, :],
            tempv[0:128, None],
            tempv[128:256, None],
        ],
        axis=1,
    ).astype(np.float32)

    wq = np.ascontiguousarray(
        W[:, 0:C].reshape(2, P, C).transpose(1, 0, 2)
    ).astype(BF16)
    wk = np.ascontiguousarray(
        W[:, C : 2 * C].reshape(2, P, C).transpose(1, 0, 2)
    ).astype(BF16)
    wvt = np.ascontiguousarray(
        W[:, 2 * C : 3 * C].T.reshape(2, P, C).transpose(1, 0, 2)
    ).astype(BF16)

    in_maps = []
    x8t_cache = {}
    for core in range(NCORES):
        b = core // 2
        s = core % 2
        if b not in x8t_cache:
            xs = x[b].reshape(C, N)
            # [p, ko, j, cc] = xs[cc, 256j + 128ko + p]
            x8t_cache[b] = np.ascontiguousarray(
                xs.reshape(C, NPAIRS, 2, P).transpose(3, 2, 1, 0)
            ).astype(FP8).reshape(P, 2, N)
        xs = x[b].reshape(C, N)[:, s * NHALF : (s + 1) * NHALF]
        xbf = np.ascontiguousarray(
            xs.reshape(2, P, NHALF).transpose(1, 0, 2)
        ).astype(BF16)
        in_maps.append(
            {
                "x8t": x8t_cache[b],
                "xbf": xbf,
                "wq": wq,
                "wk": wk,
                "wvt": wvt,
                "consts": consts,
            }
        )

    nc = _get_program()
    trace = bool(int(os.environ.get("KERNEL_TRACE", "0")))
    res = run_bass_kernel_spmd(
        nc, in_maps, core_ids=list(range(NCORES)), trace=trace
    )
    LAST_RESULTS = res

    out_full = np.empty((B, C, N), np.float32)
    for core in range(NCORES):
        b = core // 2
        s = core % 2
        o = res.results[core]["out"].astype(np.float32)  # [128, 2, 6912]
        out_full[b][:, s * NHALF : (s + 1) * NHALF] = o.transpose(1, 0, 2).reshape(
            C, NHALF
        )
    return out_full.reshape(B, C, 24, 24, 24)
